# revision 44
# baseline (speedup 1.0000x reference)
"""Trainium2 Bass kernel for nn_BoundaryLoss (BCE over 3x3 boundary maps), v2.

Self-contained: hardcodes shapes [8,2,1024,1024] pred f32 / [8,1024,1024]
target int64-or-int32. Shards batch across 8 NeuronCores (1 image/core).

Math: with 2 classes both class-loops of the reference produce the SAME
boundary map b (values {0,1}); after the remove-long-lines kill the map is
2*b or 0. BCE-with-logits mean then reduces to a closed form over four
per-image statistics:
    kill_p / kill_t : any column-sum(over H) of b >= 300
    cp              : count of b_p
    cpt             : count of b_p AND b_t
Per image (n = H*W):
    kill_p           -> S = n*ln2
    else             -> S = (n-cp)*ln2 + cp*(2+log1p(e^-2)) - 4*(kill_t ? 0 : cpt)
loss = sum(S) / (B*n)

v2 device pipeline (graded slabs [32,64,96,126x6,76]; partitions = H rows;
every per-slab stage is split at a column seam into independent half-chains):
  DMA: pred ch0 halves on the sync HWDGE queue, ch1 halves on the scalar
       HWDGE queue, target via SWDGE cast-DMA (i32->bf16 during transfer,
       written straight into the mask tile). Consts ride the sync queue
       behind slab0's loads. Small slabs first so compute starts early.
  DVE:  m_p = (pred1 > pred0) -> bf16 per half; S = m[j-1]+m[j+1] per half
        (2x mode) + 1-col edge fixups; b = (sq <= 12.5) via tensor_scalar
        (4x mode); prod = b_p*b_t (2x)
  PE:   C = band@S + band@m per 512-half into a [126,1024] f32 PSUM tile;
        colsum rows via ones-matmuls accumulated over all slabs in PSUM
        (p/t in one PSUM tile, prod in another so output copies start
        before the last prod matmul; colsum matmuls deferred one slab)
  ACT:  sq = Square(C - 4.5) over the full [126,1024] (bias from const tile)
Boundary: replicate padding is equivalent to the reference's in-bounds
pooling for the 0<C<9 test (C==0 iff all-0 window, ==9 iff all-1).
Perf notes: ~74us on core0 (from 128us baseline). The slab grading is a
sharp empirical optimum -- small perturbations (9 uniform slabs, smoother
ramps) measured 90-120us, mostly via multi-microsecond Pool DRAIN stalls
(full-SWDGE-queue WAR fences) or DMA-queue backlog. DMA throughput is
~110 GB/s per queue sustained; one queue's first large transfer runs far
below that, hence the graded head.
"""
import math
from contextlib import ExitStack

import numpy as np
import ml_dtypes

import concourse.bass as bass
import concourse.bacc as bacc
import concourse.mybir as mybir
import concourse.tile as tile
from concourse.bass_utils import run_bass_kernel_spmd

BF16 = mybir.dt.bfloat16
FP8 = mybir.dt.float8e4
F32 = mybir.dt.float32
I32 = mybir.dt.int32

B, H, W = 8, 1024, 1024
NPIX = H * W
THR = 300.0
LN2 = math.log(2.0)
C2 = math.log1p(math.exp(-2.0))

# graded slab sizes: small first so the first compute starts as soon as a
# small DMA lands, then full 126-row slabs; input rows include +-1 halo
SLAB_SIZES = [32, 64, 96] + [126] * 6 + [76]
assert sum(SLAB_SIZES) == H
SLABS = []
o0 = 0
for sz in SLAB_SIZES:
    o1 = o0 + sz
    SLABS.append((o0, o1, max(0, o0 - 1), min(H, o1 + 1)))
    o0 = o1
NSLAB = len(SLABS)

CFG = {
    # engine for the int32->bf16 target cast: "gpsimd" | "scalar" | "vector"
    "cast": "dma",
    # engine for b = (sq <= 12.5): "vector" | "gpsimd"
    "cmp": "vector",
    # m_p / m_t edge-column replication: "gpsimd" | "vector"
    "edges_p": "gpsimd",
    "edges_t": "gpsimd",
    # queue for the target DMA chunks: "scalar" (HWDGE) | "gpsimd" (SWDGE) | "sync"
    "tgt_queue": "gpsimd",
    # queue for pred channel 1: "sync" | "scalar"
    "pred_q1": "scalar",
    # row-chunks per slab DMA (engage multiple DMA rings per transfer)
    "chunks_pred": 1,
    "chunks_tgt": 1,
    # ACT square width: 1024 (span 2 PSUM banks) or 512
    "act_w": 1024,
    # b computation: "square" (ACT Square + DVE is_le) or "mod" (single DVE
    # tensor_scalar: b = ((C mod 9) >= 0.5), reading PSUM f32 directly).
    # C is an exact integer 0..9, so mod 9 maps {0,9}->0 and 1..8->C.
    "bmode": "square",
    # cpt via "mul" (DVE mul + ones-matmul) or "ttr" (DVE tensor_tensor_reduce)
    "cpt": "mul",
    # 3-column horizontal sum: "dve" (S = m[j-1]+m[j+1] adds on DVE, two
    # band matmuls) or "pe_dr" (fp8 masks; one DoubleRow matmul sums the
    # left+right planes, one regular matmul adds the center; no S adds)
    "smode": "pe_dr",
    # colsum strategy: "mm" (per-slab PE ones-matmuls) or "bacc" (DVE
    # elementwise accumulate + end matmuls)
    "colsum": "mm",
    # compute the prod/cpt stream (only consulted when neither map is
    # killed; the host falls back to the prod build when it needs cpt)
    "prod": False,
    # SWDGE descriptor-ring carveout bytes (default 16384; bigger ring =
    # more outstanding SWDGE cast-DMAs before DRAIN fences)
    "swdge_ring": 65536,
    # pre-tile warmup DMA size per queue in KB (0 = off)
    "warm_kb": 0,
    # number of head slabs whose inputs are preloaded by pre-tile DMAs
    "preload": 0,
    # pred DMA granularity: True = column-halved transfers, False = full-width
    "dma_halves": False,
    # tile pool depths
    "pc_bufs": 8,
    "psc_bufs": 2,
    # separate PSUM tiles for p/t vs prod colsum accumulators (lets the
    # p/t output copies start before the last prod matmul; needs psc_bufs=2)
    "split_cs": True,
    "mask_bufs": 10,
    "b_bufs": 4,
    "s_bufs": 6,
    "sq_bufs": 4,
    # defer colsum matmuls by one slab (decouple PSUM accum from band MMs)
    "defer": True,
}


def _chunks(n, k):
    """Split rows [0, n) into k roughly equal contiguous pieces."""
    out = []
    step = (n + k - 1) // k
    r = 0
    while r < n:
        out.append((r, min(n, r + step)))
        r += step
    return out


def _build_band3():
    """band3 [128, 3*128] bf16: vertical replicate-pad weights for
    top / interior / bottom slabs. All interior slabs share one diagonal
    block (band[m+1+dr, m]); a smaller slab just uses its leading slice."""
    band = np.zeros((128, 3 * 128), np.float32)
    for blk, (o0, o1, i0, i1) in ((0, SLABS[0]), (2, SLABS[-1])):
        for m in range(o1 - o0):
            r = o0 + m
            for dr in (-1, 0, 1):
                rr = min(H - 1, max(0, r + dr))
                band[rr - i0, 128 * blk + m] += 1.0
    for m in range(126):  # interior: i0 = o0-1, no clamping
        for dr in (-1, 0, 1):
            band[m + 1 + dr, 128 + m] += 1.0
    return band.astype(ml_dtypes.bfloat16)


def _build_band6():
    """band6 [128, 2*384] fp8e4: two side-by-side copies of band3 (the
    DoubleRow lhsT planes). Values <= 2, exact in e4m3."""
    b3 = _build_band3().astype(np.float32)
    return np.concatenate([b3, b3], 1).astype(ml_dtypes.float8_e4m3)


def _blk(si):
    return 0 if si == 0 else (2 if si == NSLAB - 1 else 1)


def _build_nc(tgt_cols, tgt_step, with_prod):
    """Build the per-core Bass program.

    tgt_cols/tgt_step: 2048/2 when target arrives as int64 (viewed as int32
    pairs; low word at even columns), 1024/1 when it arrives as int32.
    """
    nc = bacc.Bacc("TRN2", target_bir_lowering=False, debug=False,
                   dynamic_dma_scratch_size=CFG.get("swdge_ring", 16384))

    pred = nc.dram_tensor("pred", [2, H, W], F32, kind="ExternalInput").ap()
    tgt = nc.dram_tensor("tgt", [H, tgt_cols], I32, kind="ExternalInput").ap()
    band3_d = nc.dram_tensor("band3", [128, 3 * 128], BF16,
                             kind="ExternalInput").ap()
    band6_d = nc.dram_tensor("band6", [128, 2 * 3 * 128], FP8,
                             kind="ExternalInput").ap()
    ones_d = nc.dram_tensor("ones", [128, 1], BF16, kind="ExternalInput").ap()
    bias_d = nc.dram_tensor("bias", [128, 1], F32, kind="ExternalInput").ap()
    colsums_o = nc.dram_tensor("colsums", [3, W], F32, kind="ExternalOutput").ap()

    AT = mybir.AluOpType
    AF = mybir.ActivationFunctionType

    eng = {"gpsimd": nc.gpsimd, "vector": nc.vector, "scalar": nc.scalar}
    dmaq = {"sync": nc.sync, "scalar": nc.scalar, "gpsimd": nc.gpsimd}

    if CFG["warm_kb"]:
        # fire-and-forget pre-tile DMAs: issued before the tile prologue's
        # ~7us of sem handshakes + table loads, so the DMA engines/path are
        # warm when the first real transfers issue. Nothing reads `warm`.
        rows = CFG["warm_kb"] * 256 // W  # f32 elems
        warm = nc.alloc_sbuf_tensor("warm", [128, W], F32)
        warm_sem = nc.alloc_semaphore("warm_sem")
        nc.sync.dma_start(warm.ap()[0:rows, :],
                          pred[0, 0:rows, :]).then_inc(warm_sem, 16)
        nc.scalar.dma_start(warm.ap()[0:rows, :],
                            pred[1, 0:rows, :]).then_inc(warm_sem, 16)
        nc.gpsimd.dma_start(warm.ap()[0:rows, :],
                            pred[0, H - rows:H, :]).then_inc(warm_sem, 16)

    # Preload the first PRELOAD slabs' inputs + the consts with pre-tile
    # DMAs issued BEFORE the TileContext prologue (~7us of sem handshakes +
    # ucode/ACT table loads): the transfers land during the prologue, so
    # compute starts immediately after it instead of waiting out the whole
    # issue + transfer chain. Consumers inside the tile region are gated by
    # per-engine wait_ge on pre_sem.
    n_pre = CFG["preload"]
    pre_sem = nc.alloc_semaphore("pre_sem") if n_pre else None
    pre_cnt = 0
    pre_p, pre_t = [], []
    for si in range(n_pre):
        o0, o1, i0, i1 = SLABS[si]
        n_in = i1 - i0
        pp = nc.alloc_sbuf_tensor(f"pre_p{si}", [n_in, 2 * W], F32)
        nc.sync.dma_start(pp.ap()[0:n_in, 0:W],
                          pred[0, i0:i1, :]).then_inc(pre_sem, 16)
        nc.scalar.dma_start(pp.ap()[0:n_in, W:2 * W],
                            pred[1, i0:i1, :]).then_inc(pre_sem, 16)
        mt = nc.alloc_sbuf_tensor(f"pre_t{si}", [n_in, W + 2], BF16)
        nc.gpsimd.dma_start(mt.ap()[0:n_in, 1:W + 1],
                            tgt[i0:i1, 0:tgt_cols:tgt_step]
                            ).then_inc(pre_sem, 16)
        pre_p.append(pp)
        pre_t.append(mt)
        pre_cnt += 48
    if n_pre:
        band3_pre = nc.alloc_sbuf_tensor("band3_pre", [128, 3 * 128], BF16)
        ones_pre = nc.alloc_sbuf_tensor("ones_pre", [128, 1], BF16)
        bias_pre = nc.alloc_sbuf_tensor("bias_pre", [128, 1], F32)
        nc.sync.dma_start(band3_pre.ap()[:], band3_d).then_inc(pre_sem, 16)
        nc.scalar.dma_start(ones_pre.ap()[:], ones_d).then_inc(pre_sem, 16)
        nc.scalar.dma_start(bias_pre.ap()[:], bias_d).then_inc(pre_sem, 16)
        pre_cnt += 48
        # gate every engine on the pre-tile transfers BEFORE the tile
        # region (the tile scheduler's sim can't see pre-tile sem incs);
        # transfers overlap the engines' own prologue work either way
        for e in (nc.vector, nc.tensor, nc.scalar, nc.gpsimd, nc.sync):
            e.wait_ge(pre_sem, pre_cnt)

    with tile.TileContext(nc) as tc, ExitStack() as ctx:
        const_pool = ctx.enter_context(tc.tile_pool(name="const", bufs=1))
        pc_pool = ctx.enter_context(tc.tile_pool(name="pc", bufs=CFG["pc_bufs"]))
        tgt_pool = ctx.enter_context(tc.tile_pool(name="tgt", bufs=4))
        mask_pool = ctx.enter_context(tc.tile_pool(name="mask", bufs=CFG["mask_bufs"]))
        s_pool = ctx.enter_context(tc.tile_pool(
            name="s", bufs=1 if CFG["smode"] == "pe_dr" else CFG["s_bufs"]))
        sq_pool = ctx.enter_context(tc.tile_pool(name="sq", bufs=CFG["sq_bufs"]))
        b_pool = ctx.enter_context(tc.tile_pool(name="b", bufs=CFG["b_bufs"]))
        prod_pool = ctx.enter_context(tc.tile_pool(name="prod", bufs=1))
        out_pool = ctx.enter_context(tc.tile_pool(name="out", bufs=1))
        bacc_pool = ctx.enter_context(tc.tile_pool(name="bacc", bufs=1))
        psc_bufs = CFG["psc_bufs"]
        psum_c = ctx.enter_context(tc.tile_pool(name="psc", bufs=psc_bufs, space="PSUM"))
        psum_cs = ctx.enter_context(tc.tile_pool(name="pscs", bufs=1, space="PSUM"))

        hw = W // 2
        mA, mB = 1 + hw + 1, W + 1  # m-col seam: A writes [1, mA), B [mA, mB)
        wA, wB = mA - 1, mB - mA    # 513 / 511 image columns
        HALVES = ((0, wA), (wA, wB))
        dma_cast = CFG["cast"] == "dma" and tgt_step == 1
        pe_dr = CFG["smode"] == "pe_dr"
        MDT = FP8 if pe_dr else BF16

        def emit_slab_dma(si):
            o0, o1, i0, i1 = SLABS[si]
            n_in = i1 - i0
            pch = {}
            if not CFG["dma_halves"]:
                for c in (0, 1):
                    q = nc.sync if CFG["pred_q1"] == "sync" or c == 0 \
                        else dmaq[CFG["pred_q1"]]
                    pc = pc_pool.tile([128, W], F32, tag=f"pcf{c}")
                    q.dma_start(pc[0:n_in, :], pred[c, i0:i1, :])
                    pch[(c, 2)] = pc
                    for hi, (c0, cw) in enumerate(HALVES):
                        pch[(c, hi)] = pc[:, c0:c0 + cw]
            else:
                for c in (0, 1):
                    for hi, (c0, cw) in enumerate(HALVES):
                        if CFG["pred_q1"] == "mixed22":
                            q = nc.sync if hi == 0 else nc.scalar
                        elif CFG["pred_q1"] == "mixed":
                            q = nc.sync if (c, hi) != (1, 1) else nc.scalar
                        else:
                            q = nc.sync if CFG["pred_q1"] == "sync" or c == 0 \
                                else dmaq[CFG["pred_q1"]]
                        pc = pc_pool.tile([128, wA], F32, tag=f"pc{c}{hi}")
                        q.dma_start(pc[0:n_in, 0:cw],
                                    pred[c, i0:i1, c0:c0 + cw])
                        pch[(c, hi)] = pc
            m_t = mask_pool.tile([128, W + 2], MDT, tag="mt")
            t32h = []
            if dma_cast and not CFG.get("tgt_halves", True):
                nc.gpsimd.dma_start(m_t[0:n_in, 1:W + 1], tgt[i0:i1, :])
                return pch, m_t, t32h
            for hi, (c0, cw) in enumerate(HALVES):
                if dma_cast:
                    # SWDGE casts i32->bf16 during the transfer, writing
                    # the mask tile directly: no engine cast, no staging
                    nc.gpsimd.dma_start(m_t[0:n_in, 1 + c0:1 + c0 + cw],
                                        tgt[i0:i1, c0:c0 + cw])
                else:
                    t32 = tgt_pool.tile([128, wA * tgt_step], I32,
                                        tag=f"t32{hi}")
                    if CFG["tgt_queue"] == "mixed":
                        tq = nc.sync if hi == 0 else nc.scalar
                    else:
                        tq = dmaq[CFG["tgt_queue"]]
                    tq.dma_start(
                        t32[0:n_in, 0:cw * tgt_step],
                        tgt[i0:i1, c0 * tgt_step:(c0 + cw) * tgt_step])
                    t32h.append(t32)
            return pch, m_t, t32h

        if n_pre:
            band3, ones, biasc = (band3_pre.ap(), ones_pre.ap(),
                                  bias_pre.ap())
            slab0_dma = None
        else:
            # slab 0's input DMAs go on each queue ahead of the consts (the
            # consts aren't needed until the first matmul, ~5us later)
            slab0_dma = emit_slab_dma(0)

            band3 = const_pool.tile([128, 3 * 128], BF16)
            nc.sync.dma_start(band3[:], band3_d)
            band6 = const_pool.tile([128, 2 * 3 * 128], FP8)
            nc.sync.dma_start(band6[:], band6_d)
            ones = const_pool.tile([128, 1], BF16)
            nc.sync.dma_start(ones[:], ones_d)
            biasc = const_pool.tile([128, 1], F32)
            nc.sync.dma_start(biasc[:], bias_d)

        # colsum strategy:
        #  "bacc": bf16 per-map accumulators bacc[p,j] += b_slab[p,j] on DVE
        #          (values <= NSLAB, exact), ones@bacc ONCE at the end
        #  "mm":   per-slab ones@b matmuls accumulated in PSUM (deferred one
        #          slab), as in v2
        n_maps = 3 if with_prod else 2
        use_bacc = CFG["colsum"] == "bacc"
        baccs = []
        if use_bacc:
            for mi in range(n_maps):
                ba = bacc_pool.tile([126, W], BF16, tag=f"bacc{mi}",
                                    name=f"bacc{mi}")
                nc.vector.memset(ba[:], 0.0)
                baccs.append(ba)
        else:
            cs_ab = psum_cs.tile([33, W], F32, tag="csab")
            cs_c = psum_cs.tile([1, W], F32, tag="csc") if with_prod else None
        deferred = []
        prev_deferred = []

        for si, (o0, o1, i0, i1) in enumerate(SLABS):
            n_in = i1 - i0
            n_out = o1 - o0
            start = si == 0
            stop = si == NSLAB - 1

            # column-split DMAs with a seam at hw+1 (mask col coords): the
            # A-half covers one extra column so each half of every later
            # stage depends on only one half-chain. No overlapping writes.
            if si < n_pre:
                pp = pre_p[si].ap()
                pch = {(c, hi): pp[:, c * W + c0:c * W + c0 + cw]
                       for c in (0, 1) for hi, (c0, cw) in enumerate(HALVES)}
                m_t, t32h = pre_t[si].ap(), []
                pre_cast = True
            else:
                pch, m_t, t32h = (slab0_dma if si == 0 and slab0_dma
                                  else emit_slab_dma(si))
                pre_cast = False
            m_p = mask_pool.tile([128, W + 2], MDT, tag="mp")

            # masks live at cols 1..W of a [128, W+2] tile
            for hi, (c0, cw) in enumerate(HALVES):
                nc.vector.tensor_tensor(
                    m_p[0:n_in, 1 + c0:1 + c0 + cw],
                    pch[(1, hi)][0:n_in, 0:cw],
                    pch[(0, hi)][0:n_in, 0:cw], AT.is_gt)
                if pre_cast or dma_cast:
                    pass
                elif CFG["cast"] in ("scalar", "dma"):
                    nc.scalar.copy(
                        m_t[0:n_in, 1 + c0:1 + c0 + cw],
                        t32h[hi][0:n_in, 0:cw * tgt_step:tgt_step])
                else:
                    eng[CFG["cast"]].tensor_copy(
                        m_t[0:n_in, 1 + c0:1 + c0 + cw],
                        t32h[hi][0:n_in, 0:cw * tgt_step:tgt_step])
            if pe_dr:
                # replicate image edge columns into the pad cols so the PE
                # window sums read valid values (1-col DVE copies)
                for m in (m_p, m_t):
                    nc.vector.tensor_copy(m[0:n_in, 0:1], m[0:n_in, 1:2])
                    nc.vector.tensor_copy(m[0:n_in, W + 1:W + 2],
                                          m[0:n_in, W:W + 1])
                Ss = [None, None]
            else:
                # per-map, per-half S = m[j-1]+m[j+1] (2x DVE); the wide
                # adds read the never-written edge cols 0 / W+1; the 1-col
                # fixup adds overwrite S's first/last column.
                Ss = []
                for mi, m in enumerate((m_p, m_t)):
                    S = s_pool.tile([128, W], BF16, tag=f"S{mi}")
                    nc.vector.tensor_add(
                        S[0:n_in, 0:hw], m[0:n_in, 0:hw], m[0:n_in, 2:hw + 2])
                    nc.vector.tensor_add(
                        S[0:n_in, 0:1], m[0:n_in, 1:2], m[0:n_in, 2:3])
                    nc.vector.tensor_add(
                        S[0:n_in, hw:W], m[0:n_in, hw:W],
                        m[0:n_in, hw + 2:W + 2])
                    nc.vector.tensor_add(
                        S[0:n_in, W - 1:W], m[0:n_in, W - 1:W],
                        m[0:n_in, W:W + 1])
                    Ss.append(S)

            blk0 = 128 * _blk(si)
            lhs = band3[0:n_in, blk0:blk0 + n_out]
            lhs8 = band6[0:n_in, blk0:blk0 + n_out] if pe_dr else None
            bts = []
            for mi in (0, 1):
                m, S = (m_p, m_t)[mi], Ss[mi]
                C = psum_c.tile([126, W], F32, tag="C")
                for h0 in (0, 512):
                    if pe_dr:
                        # C[:, h0:h0+512] = band@(m[j-1]+m[j+1]) via one
                        # DoubleRow matmul (planes at cols h0, h0+2) plus
                        # band@m[j] via a regular fp8 matmul
                        lhs2 = band6[0:n_in, blk0:blk0 + n_out].unsqueeze(1)
                        lhs2.ap[1] = [3 * 128, 2]
                        rhs2 = m[0:n_in, h0:h0 + 512].unsqueeze(1)
                        rhs2.ap[1] = [2, 2]
                        nc.tensor.matmul(
                            C[0:n_out, h0:h0 + 512], lhs2, rhs2,
                            start=True, stop=False,
                            perf_mode=mybir.MatmulPerfMode.DoubleRow,
                            skip_group_check=True)
                        nc.tensor.matmul(
                            C[0:n_out, h0:h0 + 512], lhs8,
                            m[0:n_in, 1 + h0:1 + h0 + 512],
                            start=False, stop=True, skip_group_check=True)
                        continue
                    nc.tensor.matmul(
                        C[0:n_out, h0:h0 + 512], lhs, S[0:n_in, h0:h0 + 512],
                        start=True, stop=False, skip_group_check=True)
                    nc.tensor.matmul(
                        C[0:n_out, h0:h0 + 512], lhs,
                        m[0:n_in, 1 + h0:1 + h0 + 512],
                        start=False, stop=True, skip_group_check=True)
                bt = b_pool.tile([126, W], BF16, tag=f"b{mi}")
                if CFG["bmode"] == "mod":
                    nc.vector.tensor_scalar(
                        bt[0:n_out, :], C[0:n_out, :], 9.0, 0.5,
                        AT.mod, AT.is_ge)
                else:
                    sq = sq_pool.tile([126, W], BF16, tag=f"sq{mi}")
                    for a0 in range(0, W, CFG["act_w"]):
                        nc.scalar.activation(
                            sq[0:n_out, a0:a0 + CFG["act_w"]],
                            C[0:n_out, a0:a0 + CFG["act_w"]],
                            AF.Square, bias=biasc[0:n_out, :], scale=1.0)
                    eng[CFG["cmp"]].tensor_single_scalar(
                        bt[0:n_out, :], sq[0:n_out, :], 12.5, AT.is_le)
                bts.append(bt)
                if use_bacc:
                    nc.vector.tensor_add(
                        baccs[mi][0:n_out, :], baccs[mi][0:n_out, :],
                        bt[0:n_out, :])
                else:
                    for h0 in (0, 512):
                        deferred.append(
                            (lambda mi=mi, h0=h0, bt=bt, n_out=n_out,
                                    start=start, stop=stop:
                             nc.tensor.matmul(
                                 cs_ab[32 * mi:32 * mi + 1, h0:h0 + 512],
                                 ones[0:n_out, :], bt[0:n_out, h0:h0 + 512],
                                 start=start, stop=stop,
                                 skip_group_check=True)))

            if with_prod:
                prod = prod_pool.tile([126, W], BF16, tag="prod")
                nc.vector.tensor_mul(
                    prod[0:n_out, :], bts[0][0:n_out, :], bts[1][0:n_out, :])
                if use_bacc:
                    nc.vector.tensor_add(
                        baccs[2][0:n_out, :], baccs[2][0:n_out, :],
                        prod[0:n_out, :])
                else:
                    for h0 in (0, 512):
                        deferred.append(
                            (lambda h0=h0, prod=prod, n_out=n_out,
                                    start=start, stop=stop:
                             nc.tensor.matmul(
                                 cs_c[0:1, h0:h0 + 512],
                                 ones[0:n_out, :], prod[0:n_out, h0:h0 + 512],
                                 start=start, stop=stop,
                                 skip_group_check=True)))

            if CFG["defer"]:
                for th in prev_deferred:
                    th()
                prev_deferred = deferred
            else:
                for th in deferred:
                    th()
            deferred = []

        for th in prev_deferred:
            th()

        # final cross-partition reduce + output staging (one row per engine,
        # concurrently), then one DMA out
        cs_sb = out_pool.tile([65, W], F32)
        if use_bacc:
            csp = psum_cs.tile([65, W], F32, tag="cs")
            for mi in range(n_maps):
                for h0 in (0, 512):
                    nc.tensor.matmul(
                        csp[32 * mi:32 * mi + 1, h0:h0 + 512], ones[0:126, :],
                        baccs[mi][0:126, h0:h0 + 512],
                        start=True, stop=True, skip_group_check=True)
            srcs = [csp[0:1, :], csp[32:33, :], csp[64:65, :]]
        else:
            srcs = [cs_ab[0:1, :], cs_ab[32:33, :],
                    cs_c[0:1, :] if with_prod else None]
        nc.scalar.copy(cs_sb[0:1, :], srcs[0])
        nc.vector.tensor_copy(cs_sb[32:33, :], srcs[1])
        if with_prod:
            nc.scalar.copy(cs_sb[64:65, :], srcs[2])
            nc.sync.dma_start(colsums_o[0:3, :], cs_sb[0:65:32, :])
        else:
            nc.sync.dma_start(colsums_o[0:2, :], cs_sb[0:33:32, :])

    nc.compile()
    return nc


_NC_CACHE = {}


def _get_nc(tgt_cols, tgt_step, with_prod):
    key = (tgt_cols, tgt_step, with_prod, tuple(sorted(CFG.items())))
    if key not in _NC_CACHE:
        _NC_CACHE[key] = _build_nc(tgt_cols, tgt_step, with_prod)
    return _NC_CACHE[key]


def _prep_inputs(pred, target):
    pred = np.asarray(pred)
    if pred.dtype != np.float32:
        pred = pred.astype(np.float32)
    pred = np.ascontiguousarray(pred)
    assert pred.shape == (B, 2, H, W), pred.shape

    target = np.asarray(target)
    assert target.shape == (B, H, W), target.shape
    if target.dtype == np.int64:
        t32 = np.ascontiguousarray(target).view(np.int32).reshape(B, H, 2 * W)
        tgt_cols, tgt_step = 2 * W, 2
    elif target.dtype == np.int32:
        t32 = np.ascontiguousarray(target)
        tgt_cols, tgt_step = W, 1
    else:
        t32 = np.ascontiguousarray(target.astype(np.int32))
        tgt_cols, tgt_step = W, 1
    return pred, t32, tgt_cols, tgt_step


def _run(pred, target, trace=False, trace_kwargs=None):
    pred, t32, tgt_cols, tgt_step = _prep_inputs(pred, target)
    with_prod = bool(CFG["prod"])
    nc = _get_nc(tgt_cols, tgt_step, with_prod)

    band3_np = _build_band3()
    band6_np = _build_band6()
    ones_np = np.ones((128, 1), ml_dtypes.bfloat16)
    bias_np = np.full((128, 1), -4.5, np.float32)
    in_maps = [
        {"pred": pred[i], "tgt": t32[i], "band3": band3_np,
         "band6": band6_np, "ones": ones_np, "bias": bias_np}
        for i in range(B)
    ]
    res = run_bass_kernel_spmd(nc, in_maps, list(range(B)), trace=trace,
                               **(trace_kwargs or {}))

    need_cpt = []
    stats = []
    for i in range(B):
        cs = np.asarray(res.results[i]["colsums"], np.float64)
        kill_p = cs[0].max() >= THR
        kill_t = cs[1].max() >= THR
        stats.append((kill_p, kill_t, cs[0].sum(),
                      cs[2].sum() if with_prod else None))
        if not kill_p and not kill_t and not with_prod:
            need_cpt.append(i)

    if need_cpt:
        # rare path: cpt actually matters for some image — rerun with the
        # prod stream enabled to get colsum_pt
        nc2 = _get_nc(tgt_cols, tgt_step, True)
        res2 = run_bass_kernel_spmd(nc2, in_maps, list(range(B)))
        for i in need_cpt:
            cs2 = np.asarray(res2.results[i]["colsums"], np.float64)
            kp, kt, cp, _ = stats[i]
            stats[i] = (kp, kt, cp, cs2[2].sum())

    total = 0.0
    for kill_p, kill_t, cp, cpt in stats:
        if kill_p:
            total += NPIX * LN2
        else:
            cptv = 0.0 if kill_t else float(cpt)
            total += (NPIX - cp) * LN2 + cp * (2.0 + C2) - 4.0 * cptv
    loss = np.float32(total / (B * NPIX))
    return loss, res


def kernel(pred, target):
    return _run(pred, target)[0]



# revision 45
# speedup vs baseline: 1.1394x; 1.1394x over previous
"""Trainium2 Bass kernel for nn_BoundaryLoss (BCE over 3x3 boundary maps), v2.

Self-contained: hardcodes shapes [8,2,1024,1024] pred f32 / [8,1024,1024]
target int64-or-int32. Shards batch across 8 NeuronCores (1 image/core).

Math: with 2 classes both class-loops of the reference produce the SAME
boundary map b (values {0,1}); after the remove-long-lines kill the map is
2*b or 0. BCE-with-logits mean then reduces to a closed form over four
per-image statistics:
    kill_p / kill_t : any column-sum(over H) of b >= 300
    cp              : count of b_p
    cpt             : count of b_p AND b_t
Per image (n = H*W):
    kill_p           -> S = n*ln2
    else             -> S = (n-cp)*ln2 + cp*(2+log1p(e^-2)) - 4*(kill_t ? 0 : cpt)
loss = sum(S) / (B*n)

v2 device pipeline (graded slabs [32,64,96,126x6,76]; partitions = H rows;
every per-slab stage is split at a column seam into independent half-chains):
  DMA: pred ch0 halves on the sync HWDGE queue, ch1 halves on the scalar
       HWDGE queue, target via SWDGE cast-DMA (i32->bf16 during transfer,
       written straight into the mask tile). Consts ride the sync queue
       behind slab0's loads. Small slabs first so compute starts early.
  DVE:  m_p = (pred1 > pred0) -> bf16 per half; S = m[j-1]+m[j+1] per half
        (2x mode) + 1-col edge fixups; b = (sq <= 12.5) via tensor_scalar
        (4x mode); prod = b_p*b_t (2x)
  PE:   C = band@S + band@m per 512-half into a [126,1024] f32 PSUM tile;
        colsum rows via ones-matmuls accumulated over all slabs in PSUM
        (p/t in one PSUM tile, prod in another so output copies start
        before the last prod matmul; colsum matmuls deferred one slab)
  ACT:  sq = Square(C - 4.5) over the full [126,1024] (bias from const tile)
Boundary: replicate padding is equivalent to the reference's in-bounds
pooling for the 0<C<9 test (C==0 iff all-0 window, ==9 iff all-1).
Perf notes: ~74us on core0 (from 128us baseline). The slab grading is a
sharp empirical optimum -- small perturbations (9 uniform slabs, smoother
ramps) measured 90-120us, mostly via multi-microsecond Pool DRAIN stalls
(full-SWDGE-queue WAR fences) or DMA-queue backlog. DMA throughput is
~110 GB/s per queue sustained; one queue's first large transfer runs far
below that, hence the graded head.
"""
import math
from contextlib import ExitStack

import numpy as np
import ml_dtypes

import concourse.bass as bass
import concourse.bacc as bacc
import concourse.mybir as mybir
import concourse.tile as tile
from concourse.bass_utils import run_bass_kernel_spmd

BF16 = mybir.dt.bfloat16
FP8 = mybir.dt.float8e4
F32 = mybir.dt.float32
I32 = mybir.dt.int32

B, H, W = 8, 1024, 1024
NPIX = H * W
THR = 300.0
LN2 = math.log(2.0)
C2 = math.log1p(math.exp(-2.0))

# graded slab sizes: small first so the first compute starts as soon as a
# small DMA lands, then full 126-row slabs; input rows include +-1 halo
SLAB_SIZES = [32, 64, 96] + [126] * 6 + [76]
assert sum(SLAB_SIZES) == H
SLABS = []
o0 = 0
for sz in SLAB_SIZES:
    o1 = o0 + sz
    SLABS.append((o0, o1, max(0, o0 - 1), min(H, o1 + 1)))
    o0 = o1
NSLAB = len(SLABS)

CFG = {
    # engine for the int32->bf16 target cast: "gpsimd" | "scalar" | "vector"
    "cast": "dma",
    # engine for b = (sq <= 12.5): "vector" | "gpsimd"
    "cmp": "vector",
    # m_p / m_t edge-column replication: "gpsimd" | "vector"
    "edges_p": "gpsimd",
    "edges_t": "gpsimd",
    # queue for the target DMA chunks: "scalar" (HWDGE) | "gpsimd" (SWDGE) | "sync"
    "tgt_queue": "gpsimd",
    # queue for pred channel 1: "sync" | "scalar"
    "pred_q1": "scalar",
    # row-chunks per slab DMA (engage multiple DMA rings per transfer)
    "chunks_pred": 1,
    "chunks_tgt": 1,
    # ACT square width: 1024 (span 2 PSUM banks) or 512
    "act_w": 1024,
    # b computation: "square" (ACT Square + DVE is_le) or "mod" (single DVE
    # tensor_scalar: b = ((C mod 9) >= 0.5), reading PSUM f32 directly).
    # C is an exact integer 0..9, so mod 9 maps {0,9}->0 and 1..8->C.
    "bmode": "square",
    # cpt via "mul" (DVE mul + ones-matmul) or "ttr" (DVE tensor_tensor_reduce)
    "cpt": "mul",
    # 3-column horizontal sum: "dve" (S = m[j-1]+m[j+1] adds on DVE, two
    # band matmuls) or "pe_dr" (fp8 masks; one DoubleRow matmul sums the
    # left+right planes, one regular matmul adds the center; no S adds)
    "smode": "dve",
    # colsum strategy: "mm" (per-slab PE ones-matmuls) or "bacc" (DVE
    # elementwise accumulate + end matmuls)
    "colsum": "mm",
    # compute the prod/cpt stream (only consulted when neither map is
    # killed; the host falls back to the prod build when it needs cpt)
    "prod": False,
    # SWDGE descriptor-ring carveout bytes (default 16384; bigger ring =
    # more outstanding SWDGE cast-DMAs before DRAIN fences)
    "swdge_ring": 65536,
    # pre-tile warmup DMA size per queue in KB (0 = off)
    "warm_kb": 0,
    # number of head slabs whose inputs are preloaded by pre-tile DMAs
    "preload": 0,
    # pred DMA granularity: True = column-halved transfers, False = full-width
    "dma_halves": False,
    # tile pool depths
    "pc_bufs": 6,
    "psc_bufs": 3,
    # separate PSUM tiles for p/t vs prod colsum accumulators (lets the
    # p/t output copies start before the last prod matmul; needs psc_bufs=2)
    "split_cs": True,
    "mask_bufs": 6,
    "b_bufs": 4,
    "s_bufs": 6,
    "sq_bufs": 4,
    # defer colsum matmuls by one slab (decouple PSUM accum from band MMs)
    "defer": True,
}


def _chunks(n, k):
    """Split rows [0, n) into k roughly equal contiguous pieces."""
    out = []
    step = (n + k - 1) // k
    r = 0
    while r < n:
        out.append((r, min(n, r + step)))
        r += step
    return out


def _build_band3():
    """band3 [128, 3*128] bf16: vertical replicate-pad weights for
    top / interior / bottom slabs. All interior slabs share one diagonal
    block (band[m+1+dr, m]); a smaller slab just uses its leading slice."""
    band = np.zeros((128, 3 * 128), np.float32)
    for blk, (o0, o1, i0, i1) in ((0, SLABS[0]), (2, SLABS[-1])):
        for m in range(o1 - o0):
            r = o0 + m
            for dr in (-1, 0, 1):
                rr = min(H - 1, max(0, r + dr))
                band[rr - i0, 128 * blk + m] += 1.0
    for m in range(126):  # interior: i0 = o0-1, no clamping
        for dr in (-1, 0, 1):
            band[m + 1 + dr, 128 + m] += 1.0
    return band.astype(ml_dtypes.bfloat16)


def _build_band6():
    """band6 [128, 2*384] fp8e4: two side-by-side copies of band3 (the
    DoubleRow lhsT planes). Values <= 2, exact in e4m3."""
    b3 = _build_band3().astype(np.float32)
    return np.concatenate([b3, b3], 1).astype(ml_dtypes.float8_e4m3)


def _blk(si):
    return 0 if si == 0 else (2 if si == NSLAB - 1 else 1)


def _build_nc(tgt_cols, tgt_step, with_prod):
    """Build the per-core Bass program.

    tgt_cols/tgt_step: 2048/2 when target arrives as int64 (viewed as int32
    pairs; low word at even columns), 1024/1 when it arrives as int32.
    """
    nc = bacc.Bacc("TRN2", target_bir_lowering=False, debug=False,
                   dynamic_dma_scratch_size=CFG.get("swdge_ring", 16384))

    pred = nc.dram_tensor("pred", [2, H, W], F32, kind="ExternalInput").ap()
    tgt = nc.dram_tensor("tgt", [H, tgt_cols], I32, kind="ExternalInput").ap()
    band3_d = nc.dram_tensor("band3", [128, 3 * 128], BF16,
                             kind="ExternalInput").ap()
    band6_d = nc.dram_tensor("band6", [128, 2 * 3 * 128], FP8,
                             kind="ExternalInput").ap()
    ones_d = nc.dram_tensor("ones", [128, 1], BF16, kind="ExternalInput").ap()
    bias_d = nc.dram_tensor("bias", [128, 1], F32, kind="ExternalInput").ap()
    colsums_o = nc.dram_tensor("colsums", [3, W], F32, kind="ExternalOutput").ap()

    AT = mybir.AluOpType
    AF = mybir.ActivationFunctionType

    eng = {"gpsimd": nc.gpsimd, "vector": nc.vector, "scalar": nc.scalar}
    dmaq = {"sync": nc.sync, "scalar": nc.scalar, "gpsimd": nc.gpsimd}

    if CFG["warm_kb"]:
        # fire-and-forget pre-tile DMAs: issued before the tile prologue's
        # ~7us of sem handshakes + table loads, so the DMA engines/path are
        # warm when the first real transfers issue. Nothing reads `warm`.
        rows = CFG["warm_kb"] * 256 // W  # f32 elems
        warm = nc.alloc_sbuf_tensor("warm", [128, W], F32)
        warm_sem = nc.alloc_semaphore("warm_sem")
        nc.sync.dma_start(warm.ap()[0:rows, :],
                          pred[0, 0:rows, :]).then_inc(warm_sem, 16)
        nc.scalar.dma_start(warm.ap()[0:rows, :],
                            pred[1, 0:rows, :]).then_inc(warm_sem, 16)
        nc.gpsimd.dma_start(warm.ap()[0:rows, :],
                            pred[0, H - rows:H, :]).then_inc(warm_sem, 16)

    # Preload the first PRELOAD slabs' inputs + the consts with pre-tile
    # DMAs issued BEFORE the TileContext prologue (~7us of sem handshakes +
    # ucode/ACT table loads): the transfers land during the prologue, so
    # compute starts immediately after it instead of waiting out the whole
    # issue + transfer chain. Consumers inside the tile region are gated by
    # per-engine wait_ge on pre_sem.
    n_pre = CFG["preload"]
    pre_sem = nc.alloc_semaphore("pre_sem") if n_pre else None
    pre_cnt = 0
    pre_p, pre_t = [], []
    for si in range(n_pre):
        o0, o1, i0, i1 = SLABS[si]
        n_in = i1 - i0
        pp = nc.alloc_sbuf_tensor(f"pre_p{si}", [n_in, 2 * W], F32)
        nc.sync.dma_start(pp.ap()[0:n_in, 0:W],
                          pred[0, i0:i1, :]).then_inc(pre_sem, 16)
        nc.scalar.dma_start(pp.ap()[0:n_in, W:2 * W],
                            pred[1, i0:i1, :]).then_inc(pre_sem, 16)
        mt = nc.alloc_sbuf_tensor(f"pre_t{si}", [n_in, W + 2], BF16)
        nc.gpsimd.dma_start(mt.ap()[0:n_in, 1:W + 1],
                            tgt[i0:i1, 0:tgt_cols:tgt_step]
                            ).then_inc(pre_sem, 16)
        pre_p.append(pp)
        pre_t.append(mt)
        pre_cnt += 48
    if n_pre:
        band3_pre = nc.alloc_sbuf_tensor("band3_pre", [128, 3 * 128], BF16)
        ones_pre = nc.alloc_sbuf_tensor("ones_pre", [128, 1], BF16)
        bias_pre = nc.alloc_sbuf_tensor("bias_pre", [128, 1], F32)
        nc.sync.dma_start(band3_pre.ap()[:], band3_d).then_inc(pre_sem, 16)
        nc.scalar.dma_start(ones_pre.ap()[:], ones_d).then_inc(pre_sem, 16)
        nc.scalar.dma_start(bias_pre.ap()[:], bias_d).then_inc(pre_sem, 16)
        pre_cnt += 48
        # gate every engine on the pre-tile transfers BEFORE the tile
        # region (the tile scheduler's sim can't see pre-tile sem incs);
        # transfers overlap the engines' own prologue work either way
        for e in (nc.vector, nc.tensor, nc.scalar, nc.gpsimd, nc.sync):
            e.wait_ge(pre_sem, pre_cnt)

    with tile.TileContext(nc) as tc, ExitStack() as ctx:
        const_pool = ctx.enter_context(tc.tile_pool(name="const", bufs=1))
        pc_pool = ctx.enter_context(tc.tile_pool(name="pc", bufs=CFG["pc_bufs"]))
        tgt_pool = ctx.enter_context(tc.tile_pool(name="tgt", bufs=4))
        mask_pool = ctx.enter_context(tc.tile_pool(name="mask", bufs=CFG["mask_bufs"]))
        s_pool = ctx.enter_context(tc.tile_pool(
            name="s", bufs=1 if CFG["smode"] == "pe_dr" else CFG["s_bufs"]))
        sq_pool = ctx.enter_context(tc.tile_pool(name="sq", bufs=CFG["sq_bufs"]))
        b_pool = ctx.enter_context(tc.tile_pool(name="b", bufs=CFG["b_bufs"]))
        prod_pool = ctx.enter_context(tc.tile_pool(name="prod", bufs=1))
        out_pool = ctx.enter_context(tc.tile_pool(name="out", bufs=1))
        bacc_pool = ctx.enter_context(tc.tile_pool(name="bacc", bufs=1))
        psc_bufs = CFG["psc_bufs"]
        psum_c = ctx.enter_context(tc.tile_pool(name="psc", bufs=psc_bufs, space="PSUM"))
        psum_cs = ctx.enter_context(tc.tile_pool(name="pscs", bufs=1, space="PSUM"))

        hw = W // 2
        mA, mB = 1 + hw + 1, W + 1  # m-col seam: A writes [1, mA), B [mA, mB)
        wA, wB = mA - 1, mB - mA    # 513 / 511 image columns
        HALVES = ((0, wA), (wA, wB))
        dma_cast = CFG["cast"] == "dma" and tgt_step == 1
        pe_dr = CFG["smode"] == "pe_dr"
        MDT = FP8 if pe_dr else BF16

        def emit_slab_dma(si):
            o0, o1, i0, i1 = SLABS[si]
            n_in = i1 - i0
            pch = {}
            if not CFG["dma_halves"]:
                for c in (0, 1):
                    q = nc.sync if CFG["pred_q1"] == "sync" or c == 0 \
                        else dmaq[CFG["pred_q1"]]
                    pc = pc_pool.tile([128, W], F32, tag=f"pcf{c}")
                    q.dma_start(pc[0:n_in, :], pred[c, i0:i1, :])
                    pch[(c, 2)] = pc
                    for hi, (c0, cw) in enumerate(HALVES):
                        pch[(c, hi)] = pc[:, c0:c0 + cw]
            else:
                for c in (0, 1):
                    for hi, (c0, cw) in enumerate(HALVES):
                        if CFG["pred_q1"] == "mixed22":
                            q = nc.sync if hi == 0 else nc.scalar
                        elif CFG["pred_q1"] == "mixed":
                            q = nc.sync if (c, hi) != (1, 1) else nc.scalar
                        else:
                            q = nc.sync if CFG["pred_q1"] == "sync" or c == 0 \
                                else dmaq[CFG["pred_q1"]]
                        pc = pc_pool.tile([128, wA], F32, tag=f"pc{c}{hi}")
                        q.dma_start(pc[0:n_in, 0:cw],
                                    pred[c, i0:i1, c0:c0 + cw])
                        pch[(c, hi)] = pc
            m_t = mask_pool.tile([128, W + 2], MDT, tag="mt")
            t32h = []
            if dma_cast and not CFG.get("tgt_halves", True):
                nc.gpsimd.dma_start(m_t[0:n_in, 1:W + 1], tgt[i0:i1, :])
                return pch, m_t, t32h
            for hi, (c0, cw) in enumerate(HALVES):
                if dma_cast:
                    # SWDGE casts i32->bf16 during the transfer, writing
                    # the mask tile directly: no engine cast, no staging
                    nc.gpsimd.dma_start(m_t[0:n_in, 1 + c0:1 + c0 + cw],
                                        tgt[i0:i1, c0:c0 + cw])
                else:
                    t32 = tgt_pool.tile([128, wA * tgt_step], I32,
                                        tag=f"t32{hi}")
                    if CFG["tgt_queue"] == "mixed":
                        tq = nc.sync if hi == 0 else nc.scalar
                    else:
                        tq = dmaq[CFG["tgt_queue"]]
                    tq.dma_start(
                        t32[0:n_in, 0:cw * tgt_step],
                        tgt[i0:i1, c0 * tgt_step:(c0 + cw) * tgt_step])
                    t32h.append(t32)
            return pch, m_t, t32h

        if n_pre:
            band3, ones, biasc = (band3_pre.ap(), ones_pre.ap(),
                                  bias_pre.ap())
            slab0_dma = None
        else:
            # slab 0's input DMAs go on each queue ahead of the consts (the
            # consts aren't needed until the first matmul, ~5us later)
            slab0_dma = emit_slab_dma(0)

            band3 = const_pool.tile([128, 3 * 128], BF16)
            nc.sync.dma_start(band3[:], band3_d)
            band6 = const_pool.tile([128, 2 * 3 * 128], FP8)
            nc.sync.dma_start(band6[:], band6_d)
            ones = const_pool.tile([128, 1], BF16)
            nc.sync.dma_start(ones[:], ones_d)
            biasc = const_pool.tile([128, 1], F32)
            nc.sync.dma_start(biasc[:], bias_d)

        # colsum strategy:
        #  "bacc": bf16 per-map accumulators bacc[p,j] += b_slab[p,j] on DVE
        #          (values <= NSLAB, exact), ones@bacc ONCE at the end
        #  "mm":   per-slab ones@b matmuls accumulated in PSUM (deferred one
        #          slab), as in v2
        n_maps = 3 if with_prod else 2
        use_bacc = CFG["colsum"] == "bacc"
        baccs = []
        if use_bacc:
            for mi in range(n_maps):
                ba = bacc_pool.tile([126, W], BF16, tag=f"bacc{mi}",
                                    name=f"bacc{mi}")
                nc.vector.memset(ba[:], 0.0)
                baccs.append(ba)
        else:
            cs_ab = psum_cs.tile([33, W], F32, tag="csab")
            cs_c = psum_cs.tile([1, W], F32, tag="csc") if with_prod else None
        deferred = []
        prev_deferred = []

        for si, (o0, o1, i0, i1) in enumerate(SLABS):
            n_in = i1 - i0
            n_out = o1 - o0
            start = si == 0
            stop = si == NSLAB - 1

            # column-split DMAs with a seam at hw+1 (mask col coords): the
            # A-half covers one extra column so each half of every later
            # stage depends on only one half-chain. No overlapping writes.
            if si < n_pre:
                pp = pre_p[si].ap()
                pch = {(c, hi): pp[:, c * W + c0:c * W + c0 + cw]
                       for c in (0, 1) for hi, (c0, cw) in enumerate(HALVES)}
                m_t, t32h = pre_t[si].ap(), []
                pre_cast = True
            else:
                pch, m_t, t32h = (slab0_dma if si == 0 and slab0_dma
                                  else emit_slab_dma(si))
                pre_cast = False
            m_p = mask_pool.tile([128, W + 2], MDT, tag="mp")

            # masks live at cols 1..W of a [128, W+2] tile
            for hi, (c0, cw) in enumerate(HALVES):
                nc.vector.tensor_tensor(
                    m_p[0:n_in, 1 + c0:1 + c0 + cw],
                    pch[(1, hi)][0:n_in, 0:cw],
                    pch[(0, hi)][0:n_in, 0:cw], AT.is_gt)
                if pre_cast or dma_cast:
                    pass
                elif CFG["cast"] in ("scalar", "dma"):
                    nc.scalar.copy(
                        m_t[0:n_in, 1 + c0:1 + c0 + cw],
                        t32h[hi][0:n_in, 0:cw * tgt_step:tgt_step])
                else:
                    eng[CFG["cast"]].tensor_copy(
                        m_t[0:n_in, 1 + c0:1 + c0 + cw],
                        t32h[hi][0:n_in, 0:cw * tgt_step:tgt_step])
            if pe_dr:
                # replicate image edge columns into the pad cols so the PE
                # window sums read valid values (1-col DVE copies)
                for m in (m_p, m_t):
                    nc.vector.tensor_copy(m[0:n_in, 0:1], m[0:n_in, 1:2])
                    nc.vector.tensor_copy(m[0:n_in, W + 1:W + 2],
                                          m[0:n_in, W:W + 1])
                Ss = [None, None]
            else:
                # per-map, per-half S = m[j-1]+m[j+1] (2x DVE); the wide
                # adds read the never-written edge cols 0 / W+1; the 1-col
                # fixup adds overwrite S's first/last column.
                Ss = []
                for mi, m in enumerate((m_p, m_t)):
                    S = s_pool.tile([128, W], BF16, tag=f"S{mi}")
                    nc.vector.tensor_add(
                        S[0:n_in, 0:hw], m[0:n_in, 0:hw], m[0:n_in, 2:hw + 2])
                    nc.vector.tensor_add(
                        S[0:n_in, 0:1], m[0:n_in, 1:2], m[0:n_in, 2:3])
                    nc.vector.tensor_add(
                        S[0:n_in, hw:W], m[0:n_in, hw:W],
                        m[0:n_in, hw + 2:W + 2])
                    nc.vector.tensor_add(
                        S[0:n_in, W - 1:W], m[0:n_in, W - 1:W],
                        m[0:n_in, W:W + 1])
                    Ss.append(S)

            blk0 = 128 * _blk(si)
            lhs = band3[0:n_in, blk0:blk0 + n_out]
            lhs8 = band6[0:n_in, blk0:blk0 + n_out] if pe_dr else None
            bts = []
            for mi in (0, 1):
                m, S = (m_p, m_t)[mi], Ss[mi]
                C = psum_c.tile([126, W], F32, tag="C")
                for h0 in (0, 512):
                    if pe_dr:
                        # C[:, h0:h0+512] = band@(m[j-1]+m[j+1]) via one
                        # DoubleRow matmul (planes at cols h0, h0+2) plus
                        # band@m[j] via a regular fp8 matmul
                        lhs2 = band6[0:n_in, blk0:blk0 + n_out].unsqueeze(1)
                        lhs2.ap[1] = [3 * 128, 2]
                        rhs2 = m[0:n_in, h0:h0 + 512].unsqueeze(1)
                        rhs2.ap[1] = [2, 2]
                        nc.tensor.matmul(
                            C[0:n_out, h0:h0 + 512], lhs2, rhs2,
                            start=True, stop=False,
                            perf_mode=mybir.MatmulPerfMode.DoubleRow,
                            skip_group_check=True)
                        nc.tensor.matmul(
                            C[0:n_out, h0:h0 + 512], lhs8,
                            m[0:n_in, 1 + h0:1 + h0 + 512],
                            start=False, stop=True, skip_group_check=True)
                        continue
                    nc.tensor.matmul(
                        C[0:n_out, h0:h0 + 512], lhs, S[0:n_in, h0:h0 + 512],
                        start=True, stop=False, skip_group_check=True)
                    nc.tensor.matmul(
                        C[0:n_out, h0:h0 + 512], lhs,
                        m[0:n_in, 1 + h0:1 + h0 + 512],
                        start=False, stop=True, skip_group_check=True)
                bt = b_pool.tile([126, W], BF16, tag=f"b{mi}")
                if CFG["bmode"] == "mod":
                    nc.vector.tensor_scalar(
                        bt[0:n_out, :], C[0:n_out, :], 9.0, 0.5,
                        AT.mod, AT.is_ge)
                else:
                    sq = sq_pool.tile([126, W], BF16, tag=f"sq{mi}")
                    for a0 in range(0, W, CFG["act_w"]):
                        nc.scalar.activation(
                            sq[0:n_out, a0:a0 + CFG["act_w"]],
                            C[0:n_out, a0:a0 + CFG["act_w"]],
                            AF.Square, bias=biasc[0:n_out, :], scale=1.0)
                    eng[CFG["cmp"]].tensor_single_scalar(
                        bt[0:n_out, :], sq[0:n_out, :], 12.5, AT.is_le)
                bts.append(bt)
                if use_bacc:
                    nc.vector.tensor_add(
                        baccs[mi][0:n_out, :], baccs[mi][0:n_out, :],
                        bt[0:n_out, :])
                else:
                    for h0 in (0, 512):
                        deferred.append(
                            (lambda mi=mi, h0=h0, bt=bt, n_out=n_out,
                                    start=start, stop=stop:
                             nc.tensor.matmul(
                                 cs_ab[32 * mi:32 * mi + 1, h0:h0 + 512],
                                 ones[0:n_out, :], bt[0:n_out, h0:h0 + 512],
                                 start=start, stop=stop,
                                 skip_group_check=True)))

            if with_prod:
                prod = prod_pool.tile([126, W], BF16, tag="prod")
                nc.vector.tensor_mul(
                    prod[0:n_out, :], bts[0][0:n_out, :], bts[1][0:n_out, :])
                if use_bacc:
                    nc.vector.tensor_add(
                        baccs[2][0:n_out, :], baccs[2][0:n_out, :],
                        prod[0:n_out, :])
                else:
                    for h0 in (0, 512):
                        deferred.append(
                            (lambda h0=h0, prod=prod, n_out=n_out,
                                    start=start, stop=stop:
                             nc.tensor.matmul(
                                 cs_c[0:1, h0:h0 + 512],
                                 ones[0:n_out, :], prod[0:n_out, h0:h0 + 512],
                                 start=start, stop=stop,
                                 skip_group_check=True)))

            if CFG["defer"]:
                for th in prev_deferred:
                    th()
                prev_deferred = deferred
            else:
                for th in deferred:
                    th()
            deferred = []

        for th in prev_deferred:
            th()

        # final cross-partition reduce + output staging (one row per engine,
        # concurrently), then one DMA out
        cs_sb = out_pool.tile([65, W], F32)
        if use_bacc:
            csp = psum_cs.tile([65, W], F32, tag="cs")
            for mi in range(n_maps):
                for h0 in (0, 512):
                    nc.tensor.matmul(
                        csp[32 * mi:32 * mi + 1, h0:h0 + 512], ones[0:126, :],
                        baccs[mi][0:126, h0:h0 + 512],
                        start=True, stop=True, skip_group_check=True)
            srcs = [csp[0:1, :], csp[32:33, :], csp[64:65, :]]
        else:
            srcs = [cs_ab[0:1, :], cs_ab[32:33, :],
                    cs_c[0:1, :] if with_prod else None]
        nc.scalar.copy(cs_sb[0:1, :], srcs[0])
        nc.vector.tensor_copy(cs_sb[32:33, :], srcs[1])
        if with_prod:
            nc.scalar.copy(cs_sb[64:65, :], srcs[2])
            nc.sync.dma_start(colsums_o[0:3, :], cs_sb[0:65:32, :])
        else:
            nc.sync.dma_start(colsums_o[0:2, :], cs_sb[0:33:32, :])

    nc.compile()
    return nc


_NC_CACHE = {}


def _get_nc(tgt_cols, tgt_step, with_prod):
    key = (tgt_cols, tgt_step, with_prod, tuple(sorted(CFG.items())))
    if key not in _NC_CACHE:
        _NC_CACHE[key] = _build_nc(tgt_cols, tgt_step, with_prod)
    return _NC_CACHE[key]


def _prep_inputs(pred, target):
    pred = np.asarray(pred)
    if pred.dtype != np.float32:
        pred = pred.astype(np.float32)
    pred = np.ascontiguousarray(pred)
    assert pred.shape == (B, 2, H, W), pred.shape

    target = np.asarray(target)
    assert target.shape == (B, H, W), target.shape
    if target.dtype == np.int64:
        t32 = np.ascontiguousarray(target).view(np.int32).reshape(B, H, 2 * W)
        tgt_cols, tgt_step = 2 * W, 2
    elif target.dtype == np.int32:
        t32 = np.ascontiguousarray(target)
        tgt_cols, tgt_step = W, 1
    else:
        t32 = np.ascontiguousarray(target.astype(np.int32))
        tgt_cols, tgt_step = W, 1
    return pred, t32, tgt_cols, tgt_step


def _run(pred, target, trace=False, trace_kwargs=None):
    pred, t32, tgt_cols, tgt_step = _prep_inputs(pred, target)
    with_prod = bool(CFG["prod"])
    nc = _get_nc(tgt_cols, tgt_step, with_prod)

    band3_np = _build_band3()
    band6_np = _build_band6()
    ones_np = np.ones((128, 1), ml_dtypes.bfloat16)
    bias_np = np.full((128, 1), -4.5, np.float32)
    in_maps = [
        {"pred": pred[i], "tgt": t32[i], "band3": band3_np,
         "band6": band6_np, "ones": ones_np, "bias": bias_np}
        for i in range(B)
    ]
    res = run_bass_kernel_spmd(nc, in_maps, list(range(B)), trace=trace,
                               **(trace_kwargs or {}))

    need_cpt = []
    stats = []
    for i in range(B):
        cs = np.asarray(res.results[i]["colsums"], np.float64)
        kill_p = cs[0].max() >= THR
        kill_t = cs[1].max() >= THR
        stats.append((kill_p, kill_t, cs[0].sum(),
                      cs[2].sum() if with_prod else None))
        if not kill_p and not kill_t and not with_prod:
            need_cpt.append(i)

    if need_cpt:
        # rare path: cpt actually matters for some image — rerun with the
        # prod stream enabled to get colsum_pt
        nc2 = _get_nc(tgt_cols, tgt_step, True)
        res2 = run_bass_kernel_spmd(nc2, in_maps, list(range(B)))
        for i in need_cpt:
            cs2 = np.asarray(res2.results[i]["colsums"], np.float64)
            kp, kt, cp, _ = stats[i]
            stats[i] = (kp, kt, cp, cs2[2].sum())

    total = 0.0
    for kill_p, kill_t, cp, cpt in stats:
        if kill_p:
            total += NPIX * LN2
        else:
            cptv = 0.0 if kill_t else float(cpt)
            total += (NPIX - cp) * LN2 + cp * (2.0 + C2) - 4.0 * cptv
    loss = np.float32(total / (B * NPIX))
    return loss, res


def kernel(pred, target):
    return _run(pred, target)[0]

